# revision 18
# baseline (speedup 1.0000x reference)
"""Bahdanau attention kernel for Trainium2, SPMD over 8 NeuronCores.

Reference computation (per batch b):
    pq    = query @ Wq.T                      # [U]
    vals  = keys[b] @ Wm.T                    # [T, U]
    score = tanh(pq + vals) @ Wa[0]           # [T]
    w     = softmax(score)                    # [T]
    ctx   = w @ keys[b]                       # [MS]
Outputs: (ctx [B, MS], score [B, T])

Strategy: shard batch B=64 across 8 cores (8 batches/core), weights
replicated.  All matmuls on the TensorEngine in bf16 (fp32 PSUM accum).
keys are fed pre-transposed ([B, MS, T], bf16, host-prepped layout) so the
contraction over MS sits on the partition dim; the context matmul is done
on the VectorEngine as a fused multiply+reduce over the free (T) dim so
keys are read from HBM exactly once.
"""

from contextlib import ExitStack

import numpy as np
import ml_dtypes

import concourse.bass as bass
import concourse.tile as tile
from concourse import bacc, mybir

BF16 = mybir.dt.bfloat16
F32 = mybir.dt.float32
AF = mybir.ActivationFunctionType

P = 128
B, T, U, QS, MS = 64, 2048, 512, 512, 512
NCORES = 8
BP = B // NCORES  # batches per core
NMT = MS // P     # ms tiles (contraction tiles of the vals matmul)
NUT = U // P      # u tiles
NQT = QS // P     # qs tiles
CH = 512          # t-chunk width (one PSUM bank of fp32)


def build_program(bp: int = BP, t: int = T, stage: int = 3):
    """Build the single-core Bass program (run SPMD on all cores).

    stage: 1 = phase A only, 2 = + softmax, 3 = full (context).
    """
    nch = t // CH   # t chunks
    ns = t // P     # t subtiles

    nc = bacc.Bacc(
        "TRN2", target_bir_lowering=False, debug=False, num_devices=NCORES
    )

    keysT_d = nc.dram_tensor("keysT", [bp, MS, t], BF16, kind="ExternalInput")
    qT_d = nc.dram_tensor("queryT", [QS, bp], BF16, kind="ExternalInput")
    wmt_d = nc.dram_tensor("wmt", [MS, U], BF16, kind="ExternalInput")
    wqt_d = nc.dram_tensor("wqt", [QS, U], BF16, kind="ExternalInput")
    wa_d = nc.dram_tensor("wa", [P, NUT], BF16, kind="ExternalInput")
    score_d = nc.dram_tensor("score", [bp, t], F32, kind="ExternalOutput")
    ctx_d = nc.dram_tensor("ctx", [bp, MS], F32, kind="ExternalOutput")

    with tile.TileContext(nc) as tc, ExitStack() as ctx:
        _emit_body(nc, tc, ctx, bp, t, stage, keysT_d, qT_d, wmt_d, wqt_d,
                   wa_d, score_d, ctx_d)
    nc.compile()
    return nc


def _emit_body(nc, tc, ctx, bp, t, stage, keysT_d, qT_d, wmt_d, wqt_d,
               wa_d, score_d, ctx_d):
    nch = t // CH
    if True:
        const_pool = ctx.enter_context(tc.tile_pool(name="const", bufs=1))
        psum_v = ctx.enter_context(
            tc.tile_pool(name="psv", bufs=4, space="PSUM")
        )
        psum_m = ctx.enter_context(
            tc.tile_pool(name="psm", bufs=4, space="PSUM")
        )

        # --- weights to SBUF ---
        wmt_sb = const_pool.tile([P, NMT, U], BF16, tag="wmt")
        nc.sync.dma_start(wmt_sb[:], wmt_d.ap().rearrange("(mt p) u -> p mt u", p=P))
        wa_sb = const_pool.tile([P, NUT], BF16, tag="wa")
        nc.sync.dma_start(wa_sb[:], wa_d.ap())
        ones_sb = const_pool.tile([1, P], BF16, tag="ones")
        nc.gpsimd.memset(ones_sb[:], 1.0)
        pq_sb = const_pool.tile([P, NUT * bp], F32, tag="pq")

        # --- pq = (query @ Wq.T)^T as [u, b] columns ---
        # (scoped pool so the pq-only weights free their SBUF afterwards)
        with tc.tile_pool(name="pqw", bufs=1) as pq_pool:
            wqt_sb = pq_pool.tile([P, NQT, U], BF16, tag="wqt")
            nc.sync.dma_start(
                wqt_sb[:], wqt_d.ap().rearrange("(qt p) u -> p qt u", p=P)
            )
            qt_sb = pq_pool.tile([P, NQT, bp], BF16, tag="qt")
            nc.sync.dma_start(
                qt_sb[:], qT_d.ap().rearrange("(qt p) b -> p qt b", p=P)
            )
            pq_ps = psum_m.tile([P, NUT * bp], F32, tag="ps")
            for ut in range(NUT):
                for qt in range(NQT):
                    nc.tensor.matmul(
                        pq_ps[:, ut * bp : (ut + 1) * bp],
                        wqt_sb[:, qt, bass.ts(ut, P)],
                        qt_sb[:, qt, :],
                        start=(qt == 0),
                        stop=(qt == NQT - 1),
                    )
            nc.vector.tensor_copy(pq_sb[:], pq_ps[:])

        kt_pool = ctx.enter_context(tc.tile_pool(name="kt", bufs=1))
        th_pool = ctx.enter_context(tc.tile_pool(name="th", bufs=4))
        row_pool = ctx.enter_context(tc.tile_pool(name="row", bufs=2))
        soft_pool = ctx.enter_context(tc.tile_pool(name="soft", bufs=1))
        wb_pool = ctx.enter_context(tc.tile_pool(name="wb", bufs=2))
        tmp_pool = ctx.enter_context(tc.tile_pool(name="tmp", bufs=1))

        # --- resident keysT tiles, one per batch ---
        kts = []
        for b in range(bp):
            kt = kt_pool.tile([P, NMT, t], BF16, tag=f"kt{b}")
            nc.sync.dma_start(
                kt[:], keysT_d.ap()[b].rearrange("(mt p) t -> p mt t", p=P)
            )
            kts.append(kt)

        scores_sb = soft_pool.tile([bp, t], F32, tag="scores")

        # --- phase A: vals matmul + tanh + score matmul, per batch ---
        for b in range(bp):
            kt = kts[b]
            ps = [
                psum_m.tile([1, CH], F32, tag="ps", name=f"ps{b}_{c}")
                for c in range(nch)
            ]
            for ut in range(NUT):
                pvs = [
                    psum_v.tile([P, CH], F32, tag="pv", name=f"pv{b}_{ut}_{c}")
                    for c in range(nch)
                ]
                for mt in range(NMT):
                    for c in range(nch):
                        nc.tensor.matmul(
                            pvs[c][:],
                            wmt_sb[:, mt, bass.ts(ut, P)],
                            kt[:, mt, bass.ts(c, CH)],
                            start=(mt == 0),
                            stop=(mt == NMT - 1),
                        )
                for c in range(nch):
                    th = th_pool.tile([P, CH], BF16, tag="th")
                    nc.scalar.activation(
                        th[:],
                        pvs[c][:],
                        AF.Tanh,
                        bias=pq_sb[:, ut * bp + b : ut * bp + b + 1],
                    )
                    nc.tensor.matmul(
                        ps[c][:],
                        wa_sb[:, ut : ut + 1],
                        th[:],
                        start=(ut == 0),
                        stop=(ut == NUT - 1),
                    )
            # scores of this batch -> contiguous row, then DMA to partition b
            row = row_pool.tile([1, t], F32, tag="row")
            for c in range(nch):
                nc.vector.tensor_copy(row[:, bass.ts(c, CH)], ps[c][:])
            nc.sync.dma_start(scores_sb[b : b + 1, :], row[:])

        # --- raw scores out ---
        nc.sync.dma_start(score_d.ap(), scores_sb[:])

        if stage < 2:
            return

        # --- phase B: softmax over t, batched on bp partitions ---
        maxv = soft_pool.tile([bp, 1], F32, tag="maxv")
        nc.vector.tensor_reduce(
            maxv[:], scores_sb[:], axis=mybir.AxisListType.X, op=mybir.AluOpType.max
        )
        negmax = soft_pool.tile([bp, 1], F32, tag="negmax")
        nc.vector.tensor_scalar_mul(negmax[:], maxv[:], -1.0)
        p_sb = soft_pool.tile([bp, t], BF16, tag="p")
        lsum = soft_pool.tile([bp, 1], F32, tag="lsum")
        nc.scalar.activation(
            p_sb[:], scores_sb[:], AF.Exp, bias=negmax[:], accum_out=lsum[:]
        )
        rec = soft_pool.tile([bp, 1], F32, tag="rec")
        nc.vector.reciprocal(rec[:], lsum[:])
        wnorm = soft_pool.tile([bp, t], BF16, tag="wnorm")
        nc.vector.tensor_scalar_mul(wnorm[:], p_sb[:], rec[:])

        if stage < 3:
            return

        # --- phase C: ctx[b, ms] = sum_t wnorm[b, t] * keysT[b][ms, t] ---
        ctxT = soft_pool.tile([P, NMT, bp], F32, tag="ctxT")
        for b in range(bp):
            # broadcast wnorm row b to all 128 partitions
            wrow = wb_pool.tile([1, t], BF16, tag="wrow", name=f"wrow{b}")
            nc.sync.dma_start(wrow[:], wnorm[b : b + 1, :])
            wb_sb = wb_pool.tile([P, t], BF16, tag="wb", name=f"wb{b}")
            nc.gpsimd.partition_broadcast(wb_sb[:], wrow[:])
            for mt in range(NMT):
                tmp = tmp_pool.tile([P, t], BF16, tag="tmp")
                nc.vector.tensor_tensor(
                    tmp[:], kts[b][:, mt, :], wb_sb[:], op=mybir.AluOpType.mult
                )
                nc.vector.tensor_reduce(
                    ctxT[:, mt, b : b + 1],
                    tmp[:],
                    axis=mybir.AxisListType.X,
                    op=mybir.AluOpType.add,
                )

        for b in range(bp):
            nc.sync.dma_start(
                ctx_d.ap()[b].rearrange("(mt p) -> p mt", p=P), ctxT[:, :, b]
            )


_CACHE = {}


def _get_program(bp: int = BP, t: int = T):
    key = (bp, t)
    if key not in _CACHE:
        _CACHE[key] = build_program(bp, t)
    return _CACHE[key]


def _prep_inputs(query, keys, Wq, Wm, Wa):
    """Host-side layout prep: shard over cores, cast to bf16, transpose."""
    bf = ml_dtypes.bfloat16
    wmt = np.ascontiguousarray(np.asarray(Wm, np.float32).T).astype(bf)  # [MS, U]
    wqt = np.ascontiguousarray(np.asarray(Wq, np.float32).T).astype(bf)  # [QS, U]
    wa = np.ascontiguousarray(
        np.asarray(Wa, np.float32).reshape(NUT, P).T
    ).astype(bf)  # [P, NUT]
    keysT = np.asarray(keys, np.float32).transpose(0, 2, 1)  # [B, MS, T]
    queryT = np.asarray(query, np.float32).T  # [QS, B]

    in_maps = []
    for i in range(NCORES):
        bs = slice(i * BP, (i + 1) * BP)
        in_maps.append(
            {
                "keysT": np.ascontiguousarray(keysT[bs]).astype(bf),
                "queryT": np.ascontiguousarray(queryT[:, bs]).astype(bf),
                "wmt": wmt,
                "wqt": wqt,
                "wa": wa,
            }
        )
    return in_maps


def _run(query, keys, Wq, Wm, Wa, trace=False, **kw):
    from concourse.bass_utils import run_bass_kernel_spmd

    nc = _get_program()
    in_maps = _prep_inputs(query, keys, Wq, Wm, Wa)
    res = run_bass_kernel_spmd(
        nc, in_maps, core_ids=list(range(NCORES)), trace=trace, **kw
    )
    ctx = np.concatenate(
        [res.results[i]["ctx"] for i in range(NCORES)], axis=0
    ).astype(np.float32)
    score = np.concatenate(
        [res.results[i]["score"] for i in range(NCORES)], axis=0
    ).astype(np.float32)
    return (ctx, score), res


def kernel(query, keys, Wq, Wm, Wa):
    (ctx, score), _ = _run(query, keys, Wq, Wm, Wa)
    return ctx, score
